# revision 18
# baseline (speedup 1.0000x reference)
"""LoRA Linear (T=8192, D_in=D_out=4096, r=16) on 8 TRN2 NeuronCores.

out = x @ W^T + b + (32/16) * ((x_bf16 @ A^T) @ B^T)

Strategy: data-parallel over the 8192-token axis (1024 tokens/core).
The LoRA path is folded into the dense weight on the HOST:
    W' = W + 2.0 * (B @ A)   (fp32, exact)
so the device kernel is a single dense GEMM + bias. The reference's
bf16 LoRA intermediates differ from the exact fp32 fold by ~2e-4
relative -- far below the 2e-2 gate.

Device GEMM per core: out[1024, 4096] = x^T W' + b, fp16 in / fp16 out
(PSUM accumulates fp32; host upcasts). fp16 streams 1 col/cycle like
fp32r but halves HBM traffic and enables FWL weight loads.

Schedule notes (from NTFF traces):
 - x and W are pre-interleaved on the host so every load is a single
   contiguous-per-partition 2D slice (4KB/2KB runs), halving the
   number of dma_start instructions (SP sequencer costs ~565ns each).
 - W streams on the Activation DGE queue; x/bias/stores on the SP
   queue. The DMA engines round-robin descriptors across queues, so
   isolating x from the W prefetch flood keeps the oc=0 critical path
   (JIT x loads) fed and the PE never idles long enough to re-throttle.
 - 13 dummy matmuls on a zeroed tile run during the initial DMA wait
   so the PE HAM clock-gate warms (1.2->2.4 GHz) before real work.
 - PSUM: 8 banks = one oc block of 8 token tiles; DVE adds bias on the
   PSUM->SBUF copy with fp16 output. The last oc block runs t-major so
   the drain tail collapses to one tile's copy+store.
"""

import numpy as np

try:
    import concourse  # noqa: F401
except ImportError:  # pragma: no cover
    import sys

    sys.path.insert(0, "/opt/trn_rl_repo")

from concourse import bacc, mybir, tile
from concourse.bass_utils import run_bass_kernel_spmd

N_CORES = 8
T, D_IN, D_OUT, R = 8192, 4096, 4096, 16
TPC = T // N_CORES  # 1024 tokens per core
N_DC2 = D_IN // 256  # 16 contraction chunk-pairs of 2x128
OC = 512  # output-column chunk (one PSUM bank of fp32)
N_OC = D_OUT // OC  # 8
N_TC = TPC // 128  # 8 token tiles of 128
N_WARM = 13  # HAM warmup matmuls (bridge PE from engine-init to first data)

f32 = mybir.dt.float32
f16 = mybir.dt.float16

_NC_CACHE = {}


def build_nc():
    nc = bacc.Bacc(
        "TRN2", target_bir_lowering=False, debug=False, num_devices=N_CORES
    )
    # Host-interleaved layouts (see _prepare_in_maps):
    #   xT2[p, dc2*2048 + j*1024 + t] = x[t, dc2*256 + j*128 + p]
    #   WT2[p, ((oc*16)+dc2)*1024 + j*512 + o] = W'[oc*512 + o, dc2*256 + j*128 + p]
    xT2 = nc.dram_tensor("xT2", [128, N_DC2 * 2048], f16, kind="ExternalInput").ap()
    WT2 = nc.dram_tensor(
        "WT2", [128, N_OC * N_DC2 * 1024], f16, kind="ExternalInput"
    ).ap()
    # bias is DMA'd in per-oc [128, OC] slices: a single [128, 4096] fp32
    # transfer has 16KB/partition descriptors that round-robin 1:1 with
    # the 2KB W descriptors on the shared DMA engines and starve the
    # first W tile by ~6us (measured).
    bias = nc.dram_tensor("bias", [128, D_OUT], f32, kind="ExternalInput").ap()
    out = nc.dram_tensor("out", [TPC, D_OUT], f16, kind="ExternalOutput").ap()

    with tile.TileContext(nc) as tc:
        with (
            tc.tile_pool(name="persist", bufs=1) as persist,
            tc.tile_pool(name="xpool", bufs=N_DC2) as xpool,
            tc.tile_pool(name="wpool", bufs=32) as wpool,
            tc.tile_pool(name="bpool", bufs=3) as bpool,
            tc.tile_pool(name="opool", bufs=6) as opool,
            tc.tile_pool(name="pspool", bufs=8, space="PSUM") as pspool,
        ):
            # HAM warmup: zeroed operands, scratch PSUM bank, runs while
            # the first x/W DMAs are in flight so the PE clock-gate is
            # already at 2.4 GHz when real data lands.
            wz = persist.tile([128, OC], f16, tag="wz")
            nc.vector.memset(wz[:], 0.0)
            ps_warm = pspool.tile([128, OC], f32, tag="ps", name="ps_warm")
            for _ in range(N_WARM):
                nc.tensor.matmul(
                    ps_warm[:], wz[:, 0:128], wz[:], start=True, stop=True
                )

            xt_tiles = [None] * N_DC2

            def emit_mm(ps_tiles, dc2, j, t):
                nc.tensor.matmul(
                    ps_tiles[t][:],
                    xt_tiles[dc2][
                        :, j * 1024 + t * 128 : j * 1024 + (t + 1) * 128
                    ],
                    wt_tiles[dc2][:, j * OC : (j + 1) * OC],
                    start=(dc2 == 0 and j == 0),
                    stop=(dc2 == N_DC2 - 1 and j == 1),
                )

            def emit_copy_out(ps_tiles, bias_sb, oc, t):
                osl = slice(oc * OC, (oc + 1) * OC)
                o_sb = opool.tile([128, OC], f16, tag="osb")
                nc.vector.tensor_tensor(
                    o_sb[:],
                    ps_tiles[t][:],
                    bias_sb[:],
                    mybir.AluOpType.add,
                )
                nc.sync.dma_start(
                    out=out[t * 128 : (t + 1) * 128, osl], in_=o_sb[:]
                )

            for oc in range(N_OC):
                ps_tiles = [
                    pspool.tile([128, OC], f32, tag="ps", name=f"ps_{oc}_{t}")
                    for t in range(N_TC)
                ]
                bias_sb = bpool.tile([128, OC], f32, tag="bias")
                nc.sync.dma_start(
                    out=bias_sb[:], in_=bias[:, oc * OC : (oc + 1) * OC]
                )
                wt_tiles = [None] * N_DC2
                if oc == 0:
                    # dc2-major: accumulate all 8 token tiles per W chunk,
                    # in x-arrival order (oc==0 loads x JIT).
                    for dc2 in range(N_DC2):
                        if oc == 0:
                            xt = xpool.tile([128, 2048], f16, tag="xt")
                            if dc2 == 0:
                                # split the first chunk's loads in j-halves
                                # so the first 8 matmuls start ~0.7us
                                # earlier (half the critical first bytes)
                                for j in range(2):
                                    nc.sync.dma_start(
                                        out=xt[:, j * 1024 : (j + 1) * 1024],
                                        in_=xT2[:, j * 1024 : (j + 1) * 1024],
                                    )
                            else:
                                nc.sync.dma_start(
                                    out=xt[:],
                                    in_=xT2[:, dc2 * 2048 : (dc2 + 1) * 2048],
                                )
                            xt_tiles[dc2] = xt
                        wt = wpool.tile([128, 1024], f16, tag="wt")
                        wof = (oc * N_DC2 + dc2) * 1024
                        if oc == 0 and dc2 == 0:
                            for j in range(2):
                                nc.scalar.dma_start(
                                    out=wt[:, j * OC : (j + 1) * OC],
                                    in_=WT2[:, wof + j * OC : wof + (j + 1) * OC],
                                )
                        else:
                            nc.scalar.dma_start(
                                out=wt[:], in_=WT2[:, wof : wof + 1024]
                            )
                        wt_tiles[dc2] = wt
                        for j in range(2):
                            for t in range(N_TC):
                                emit_mm(ps_tiles, dc2, j, t)
                    for t in range(N_TC):
                        emit_copy_out(ps_tiles, bias_sb, oc, t)
                else:
                    # t-major: each token tile finishes its full
                    # accumulation first, so copies/stores spread across
                    # the whole block (no end-of-block DVE/store burst),
                    # PSUM banks free ~7/8 of a block early, and the final
                    # drain collapses to one tile's copy+store. Needs the
                    # block's 16 W chunks resident (wpool=32 double-buffers
                    # against the next block's prefetch on the Act queue).
                    for dc2 in range(N_DC2):
                        wt = wpool.tile([128, 1024], f16, tag="wt")
                        wof = (oc * N_DC2 + dc2) * 1024
                        nc.scalar.dma_start(
                            out=wt[:], in_=WT2[:, wof : wof + 1024]
                        )
                        wt_tiles[dc2] = wt
                    for t in range(N_TC):
                        for dc2 in range(N_DC2):
                            for j in range(2):
                                emit_mm(ps_tiles, dc2, j, t)
                        emit_copy_out(ps_tiles, bias_sb, oc, t)

    nc.compile()
    return nc


def _prepare_in_maps(x, W, b, lora_a, lora_b):
    # Fold LoRA into the dense weight: W' = W + 2.0 * (B @ A), exact fp32.
    BA = lora_b.astype(np.float32) @ lora_a.astype(np.float32)
    Wp = W.astype(np.float32) + 2.0 * BA

    # WT2[p, (oc, dc2, j, o)] = W'[oc*512+o, dc2*256+j*128+p]
    Wt = np.ascontiguousarray(Wp.T).astype(np.float16)  # [D_IN, D_OUT]
    W4 = Wt.reshape(N_DC2, 2, 128, N_OC, OC)  # [dc2, j, p, oc, o]
    WT2 = np.ascontiguousarray(
        W4.transpose(2, 3, 0, 1, 4).reshape(128, N_OC * N_DC2 * 1024)
    )

    bias = np.ascontiguousarray(
        np.broadcast_to(b.astype(np.float32), (128, D_OUT))
    )
    in_maps = []
    for c in range(N_CORES):
        xc = x[c * TPC : (c + 1) * TPC].T.astype(np.float16)  # [D_IN, TPC]
        x4 = xc.reshape(N_DC2, 2, 128, TPC)  # [dc2, j, p, t]
        xT2 = np.ascontiguousarray(
            x4.transpose(2, 0, 1, 3).reshape(128, N_DC2 * 2048)
        )
        in_maps.append({"xT2": xT2, "WT2": WT2, "bias": bias})
    return in_maps


def run(inputs, trace=False, **trace_kwargs):
    """Run on hardware; returns (full_output, BassKernelResults)."""
    if "nc" not in _NC_CACHE:
        _NC_CACHE["nc"] = build_nc()
    nc = _NC_CACHE["nc"]
    in_maps = _prepare_in_maps(
        np.asarray(inputs["x"], dtype=np.float32),
        np.asarray(inputs["W"], dtype=np.float32),
        np.asarray(inputs["b"], dtype=np.float32),
        np.asarray(inputs["lora_a"]),
        np.asarray(inputs["lora_b"]),
    )
    res = run_bass_kernel_spmd(
        nc, in_maps, list(range(N_CORES)), trace=trace, **trace_kwargs
    )
    out = np.concatenate(
        [res.results[c]["out"] for c in range(N_CORES)], axis=0
    )
    return out.astype(np.float32), res


def kernel(**inputs):
    out, _ = run(inputs, trace=False)
    return out


if __name__ == "__main__":
    rng = np.random.default_rng(0)
    import ml_dtypes

    x = rng.standard_normal((T, D_IN), dtype=np.float32)
    W = rng.standard_normal((D_OUT, D_IN), dtype=np.float32) * 0.02
    b = rng.standard_normal((D_OUT,), dtype=np.float32) * 0.02
    la = (rng.standard_normal((R, D_IN), dtype=np.float32) * 0.02).astype(
        ml_dtypes.bfloat16
    )
    lb = (rng.standard_normal((D_OUT, R), dtype=np.float32) * 0.02).astype(
        ml_dtypes.bfloat16
    )
    got = kernel(x=x, W=W, b=b, lora_a=la, lora_b=lb)
    ref = (
        x @ W.T
        + b
        + 2.0
        * (
            (x.astype(ml_dtypes.bfloat16).astype(np.float32) @ la.astype(np.float32).T)
            @ lb.astype(np.float32).T
        )
    )
    err = np.abs(got - ref).max() / np.abs(ref).max()
    print("scale-relative max err:", err)
